# revision 26
# baseline (speedup 1.0000x reference)
"""Trainium2 Bass kernel for nn_BinaryDecorator.

Reference computation:
    x_mean = mean(|x|)                       # scalar over all of x
    out = (sign(x) @ sign(W).T + b) * x_mean # [B, OUT]

Shapes: x [65536, 512] f32, W [512, 512] f32, b [512] f32.

Strategy: data-parallel over 8 NeuronCores — shard x along batch (8192 rows
per core), replicate W and b. x_mean becomes a scalar AllReduce of per-core
sums of |x|.

Per-core dataflow (v3 — fp16 output with packed store layout):
  Phase A (streaming x, 16 groups of 4 row-tiles = 1MB per DMA):
    - DVE: row-sums of |x| via reduce_sum(apply_absolute_value)
    - PE: transpose raw f32 x tiles (via identity matmul) into PSUM
    - ACT: Sign() applied during the PSUM->SBUF copy (fp8) — this IS the
      binarize step, fused with the transpose copy
    - PE: fp8 DoubleRow matmuls per tile against pre-transposed sign(W)
    - spill raw mm (f32 PSUM) to fp16 SBUF, split ~19:13 ACT:DVE
  Phase B: partition-tree + cross-partition reduce of |x| sums, scalar
    AllReduce across the 8 cores, broadcast via ones-matmul; the 1/(B*IN)
    divide folds into an ACT scale constant (2^-25, exact).
  Phase C: one fused scalar_tensor_tensor per group:
      out_fp16 = mm * s + (b * s)   (b*s is a bf16 [128,512] tile read
    through a stride-0 broadcast AP so every operand is 2-byte -> 2x DVE).
    The DRAM output is [group, 128, 4*512] fp16 — each partition's free run
    is 4KB contiguous, so store descriptors stay >=2KB and the DMA engines
    run at full rate (1KB rows would halve them). The host undoes the
    interleave and upcasts: mm is an integer |.|<=512, so fp16 keeps the
    result to ~5e-4 relative — far inside tolerance.
"""

import sys

sys.path.insert(0, "/opt/trn_rl_repo")

import numpy as np

B, IN, OUT = 65536, 512, 512
N_CORES = 8
P = 128  # partitions
GSZ = 4  # row-tiles per group


def build_kernel(b_shard=B // N_CORES, n_cores=N_CORES):
    from concourse import bacc, bass_isa, masks, mybir, tile

    f32 = mybir.dt.float32
    f32r = mybir.dt.float32r
    f16 = mybir.dt.float16
    bf16 = mybir.dt.bfloat16
    fp8 = mybir.dt.float8e4
    AF = mybir.ActivationFunctionType
    ALU = mybir.AluOpType
    AX = mybir.AxisListType

    n_tiles = b_shard // P          # row-tiles of 128
    gsz = GSZ
    n_groups = n_tiles // gsz
    kc = IN // P                    # contraction chunks (4)
    oc = OUT // P                   # W row blocks (4)
    inv_bn = 1.0 / (B * IN)         # 2**-25, exact in f32

    nc = bacc.Bacc(
        "TRN2", target_bir_lowering=False, debug=False, num_devices=n_cores
    )
    x = nc.dram_tensor("x", [b_shard, IN], f32, kind="ExternalInput").ap()
    w = nc.dram_tensor("w", [OUT, IN], f32, kind="ExternalInput").ap()
    bias = nc.dram_tensor("b", [OUT], f32, kind="ExternalInput").ap()
    # packed store layout: [group, partition, tile-in-group * OUT] fp16
    out = nc.dram_tensor(
        "out", [n_groups, P, gsz * OUT], f16, kind="ExternalOutput"
    ).ap()

    x3 = x.rearrange("(n p) m -> n p m", p=P)      # [n_tiles, 128, 512]

    with tile.TileContext(nc) as tc:
        with (
            tc.tile_pool(name="const", bufs=1) as cpool,
            tc.tile_pool(name="mm", bufs=n_groups) as mmpool,
            tc.tile_pool(name="xg", bufs=6) as xpool,
            tc.tile_pool(name="xT", bufs=4) as xTpool,
            tc.tile_pool(name="stage", bufs=4) as stpool,
            tc.tile_pool(name="psxT", bufs=2, space="PSUM") as pxT,
            tc.tile_pool(name="psmm", bufs=2, space="PSUM") as pmm,
            tc.tile_pool(name="dram", bufs=2, space="DRAM") as dram,
        ):
            # ---- constants first: ident gates every PE transpose ----
            ident = cpool.tile([P, P], f32)
            masks.make_identity(nc, ident[:])
            identr = cpool.tile([P, P], f32r)
            nc.vector.tensor_copy(identr[:], ident[:])
            ones = cpool.tile([1, P], f32)
            nc.gpsimd.memset(ones[:], 1.0)

            # ---- warm-up collective: absorbs the ~11.5us ncfw first-call
            # wakeup so the real AllReduce's trigger latency shrinks; runs
            # entirely under phase A on the CC stream / GpSimd.
            warm = cpool.tile([1, 8], f32)
            nc.gpsimd.memset(warm[:], 0.0)
            in_w = dram.tile([1, 8], f32)
            out_w = dram.tile([1, 8], f32)
            nc.gpsimd.dma_start(in_w[:], warm[:])
            nc.gpsimd.collective_compute(
                "AllReduce",
                ALU.add,
                replica_groups=[list(range(n_cores))],
                ins=[in_w.opt()],
                outs=[out_w.opt()],
            )

            # ---- W prep: wTp[cc] [128i, 2*512o] = paired sign(W).T chunks
            # for DoubleRow matmuls. W loads go on the ACT HWDGE queue so
            # the SP queue leads with the x-tile loads.
            wtiles = []
            for j in range(oc):
                wt = cpool.tile([P, IN], f32, tag=f"wload{j}")
                nc.scalar.dma_start(wt[:], w[j * P : (j + 1) * P, :])
                wtiles.append(wt)
            wTp = [
                cpool.tile([P, 2 * OUT], fp8, tag=f"wTp{cc}", name=f"wTp{cc}")
                for cc in range(kc // 2)
            ]
            for c in range(kc):
                ps = pmm.tile([P, OUT], f32, tag="psm", name=f"wps{c}")
                for j in range(oc):
                    nc.tensor.transpose(
                        ps[:, j * P : (j + 1) * P],
                        wtiles[j][:, c * P : (c + 1) * P],
                        ident[:],
                    )
                dst = wTp[c // 2][:, (c % 2) * OUT : (c % 2 + 1) * OUT]
                nc.scalar.activation(dst, ps[:], AF.Sign)

            # ---- b prep: broadcast b across partitions ----
            b_sb = cpool.tile([1, OUT], f32)
            nc.scalar.dma_start(b_sb[:], bias[None, :])
            ps = pmm.tile([P, OUT], f32, tag="psm", name="bps")
            nc.tensor.matmul(ps[:], ones[:], b_sb[:], start=True, stop=True)
            b_bcast = cpool.tile([P, OUT], f32)
            nc.scalar.activation(b_bcast[:], ps[:], AF.Copy)

            # |x| row-sums per group land in acc columns (DVE).
            n_acc = n_groups
            acc = cpool.tile([P, n_acc], f32)

            # ---- Phase A ----
            # Software-pipelined one tile-pair deep: transposes+sign of pair
            # p are emitted before the matmuls of pair p-1, so the ACT
            # sign-copy latency hides under the next pair's PE transposes.
            # Raw matmul results are integers |.|<=512, exact in fp16.
            def emit_mms(xT, dst, p):
                # xT covers TWO row-tiles [P, 2*IN]; psm gets both results
                # side by side (two PSUM banks, one accumulation group each).
                psm = pmm.tile([P, 2 * OUT], f32, name=f"psm{p}", tag="psm")
                for tt in range(2):
                    for cc in range(kc // 2):
                        lhs = xT[
                            :, tt * IN + 2 * P * cc : tt * IN + 2 * P * (cc + 1)
                        ].rearrange("p (two m) -> p two m", two=2)
                        rhs = wTp[cc][:].rearrange(
                            "p (two n) -> p two n", two=2
                        )
                        nc.tensor.matmul(
                            psm[:, tt * OUT : (tt + 1) * OUT],
                            lhs,
                            rhs,
                            start=(cc == 0),
                            stop=(cc == kc // 2 - 1),
                            perf_mode=mybir.MatmulPerfMode.DoubleRow,
                        )
                # PSUM->SBUF fp16 spill, split ~3:2 ACT:DVE
                if p % 5 < 3:
                    nc.scalar.activation(dst, psm[:], AF.Copy)
                else:
                    nc.vector.tensor_copy(dst, psm[:])

            n_pairs = n_tiles // 2
            mm_tiles = []
            pend = None
            xg = mm_g = None
            for p in range(n_pairs):
                g, q = divmod(p, gsz // 2)
                if q == 0:
                    # xg is typed f32r so the DMA output legally feeds the
                    # f32r transposes (raw f32 bits either way); the reduce
                    # views it back as f32.
                    xg = xpool.tile(
                        [P, gsz * IN], f32r, name=f"xg{g}", tag="xg"
                    )
                    for tt in range(gsz):
                        nc.sync.dma_start(
                            xg[:, tt * IN : (tt + 1) * IN],
                            x3[g * gsz + tt].bitcast(f32r),
                        )
                    nc.vector.reduce_sum(
                        acc[:, g : g + 1], xg[:].bitcast(f32), axis=AX.X,
                        apply_absolute_value=True,
                    )
                    mm_g = mmpool.tile(
                        [P, gsz * OUT], f16, name=f"mm{g}", tag="mm"
                    )
                    mm_tiles.append(mm_g)
                # transposes run as f32r (pure 4-byte movement, 1.5 cyc/row
                # on the PE vs 2.0 for f32) via bitcast views; the Sign
                # activation reads the PSUM back as f32.
                psx = pxT.tile([P, 2 * IN], f32r, name=f"psx{p}", tag="psx")
                for tt in range(2):
                    for c in range(kc):
                        nc.tensor.transpose(
                            psx[:, tt * IN + c * P : tt * IN + (c + 1) * P],
                            xg[
                                :,
                                (2 * q + tt) * IN + c * P :
                                (2 * q + tt) * IN + (c + 1) * P,
                            ],
                            identr[:],
                        )
                xT = xTpool.tile([P, 2 * IN], fp8, name=f"xT{p}", tag="xT")
                nc.scalar.activation(xT[:], psx[:].bitcast(f32), AF.Sign)
                if pend is not None:
                    emit_mms(*pend)
                pend = (
                    xT,
                    mm_g[:, 2 * q * OUT : 2 * (q + 1) * OUT],
                    p,
                )
            emit_mms(*pend)

            # ---- Phase B: global mean of |x| ----
            # Chain on GPSIMD + SP only — both idle at the end of phase A,
            # so the AllReduce fires as soon as the last x tile has been
            # reduced, hiding the collective under the PE tail.
            acc_red = cpool.tile([P, n_acc], f32)
            nc.gpsimd.partition_all_reduce(
                acc_red[:], acc[:], channels=P, reduce_op=bass_isa.ReduceOp.add
            )
            in_b = dram.tile([1, n_acc], f32)
            out_b = dram.tile([1, n_acc], f32)
            nc.sync.dma_start(in_b[:], acc_red[:1, :])
            nc.gpsimd.collective_compute(
                "AllReduce",
                ALU.add,
                replica_groups=[list(range(n_cores))],
                ins=[in_b.opt()],
                outs=[out_b.opt()],
            )
            s_in = cpool.tile([1, n_acc], f32)
            nc.sync.dma_start(s_in[:], out_b[:])
            s_bc16 = cpool.tile([P, n_acc], f32)
            nc.gpsimd.partition_broadcast(s_bc16[:], s_in[:1, :])
            scr16 = cpool.tile([P, n_acc], f32)
            s128 = cpool.tile([P, 1], f32)
            nc.scalar.activation(
                scr16[:], s_bc16[:], AF.Copy, scale=inv_bn,
                accum_out=s128[:, :1],
            )
            # bS = (b * s) in bf16 — one small ACT op on the critical path;
            # phase C stt groups read it through a stride-0 broadcast AP.
            bS = cpool.tile([P, OUT], bf16)
            nc.scalar.activation(bS[:], b_bcast[:], AF.Copy, scale=s128[:, :1])
            bS_b = bS[:].unsqueeze(1).broadcast_to((P, gsz, OUT))

            # ---- Phase C: odd groups already hold mm+b -> one ACT scale;
            # even groups need mm*s + b*s -> DVE stt. ACT and DVE alternate
            # so the stores (one 512KB DMA per group, 4KB-per-partition
            # descriptors) stay fed at full DMA rate.
            for g in range(n_groups):
                stage = stpool.tile([P, gsz * OUT], f16)
                nc.vector.scalar_tensor_tensor(
                    out=stage[:],
                    in0=mm_tiles[g][:],
                    scalar=s128[:],
                    in1=bS_b,
                    op0=ALU.mult,
                    op1=ALU.add,
                )
                eng = nc.sync if g % 2 == 0 else nc.scalar
                eng.dma_start(out[g], stage[:])

    nc.compile()
    return nc


_CACHE = {}


def _get_runner():
    if "runner" in _CACHE:
        return _CACHE["runner"]
    import jax
    from jax.sharding import Mesh, PartitionSpec
    from jax.experimental.shard_map import shard_map
    from concourse import bass2jax, mybir

    nc = build_kernel()
    bass2jax.install_neuronx_cc_hook()
    partition_name = nc.partition_id_tensor.name if nc.partition_id_tensor else None
    in_names, out_names, out_avals = [], [], []
    for alloc in nc.m.functions[0].allocations:
        if not isinstance(alloc, mybir.MemoryLocationSet):
            continue
        name = alloc.memorylocations[0].name
        if alloc.kind == "ExternalInput":
            if name != partition_name:
                in_names.append(name)
        elif alloc.kind == "ExternalOutput":
            out_names.append(name)
            out_avals.append(
                jax.core.ShapedArray(
                    tuple(alloc.tensor_shape), mybir.dt.np(alloc.dtype)
                )
            )
    n_params = len(in_names)
    all_in_names = list(in_names) + list(out_names)
    if partition_name is not None:
        all_in_names.append(partition_name)

    def _body(*args):
        operands = list(args)
        if partition_name is not None:
            operands.append(bass2jax.partition_id_tensor())
        return tuple(
            bass2jax._bass_exec_p.bind(
                *operands,
                out_avals=tuple(out_avals),
                in_names=tuple(all_in_names),
                out_names=tuple(out_names),
                lowering_input_output_aliases=(),
                sim_require_finite=True,
                sim_require_nnan=True,
                nc=nc,
            )
        )

    devices = jax.devices()[:N_CORES]
    mesh = Mesh(np.asarray(devices), ("core",))
    n_outs = len(out_avals)
    sharded = jax.jit(
        shard_map(
            _body,
            mesh=mesh,
            in_specs=(PartitionSpec("core"),) * (n_params + n_outs),
            out_specs=(PartitionSpec("core"),) * n_outs,
            check_rep=False,
        ),
        keep_unused=True,
    )
    _CACHE["runner"] = (nc, sharded, in_names, out_names, out_avals)
    return _CACHE["runner"]


def kernel(x, W, b):
    import jax

    nc, sharded, in_names, out_names, out_avals = _get_runner()
    x = np.ascontiguousarray(x, dtype=np.float32)
    W = np.ascontiguousarray(W, dtype=np.float32)
    b = np.ascontiguousarray(b, dtype=np.float32)
    per_core = {
        "x": x,  # already concatenated along batch: shard_map splits axis 0
        "w": np.concatenate([W] * N_CORES, axis=0),
        "b": np.concatenate([b] * N_CORES, axis=0),
    }
    concat_in = [per_core[n] for n in in_names]
    concat_zeros = [
        np.zeros((N_CORES * a.shape[0], *a.shape[1:]), a.dtype) for a in out_avals
    ]
    outs = sharded(*concat_in, *concat_zeros)
    jax.block_until_ready(outs)
    res = np.asarray(outs[out_names.index("out")])
    # undo the packed store layout: [core*group, p, t, o] -> row g*512+t*128+p
    n_groups = (B // N_CORES) // (P * GSZ)
    res = res.reshape(N_CORES, n_groups, P, GSZ, OUT)
    res = res.transpose(0, 1, 3, 2, 4).reshape(B, OUT)
    return res.astype(np.float32)


if __name__ == "__main__":
    rng = np.random.default_rng(0)
    x = rng.standard_normal((B, IN)).astype(np.float32)
    W = rng.standard_normal((OUT, IN)).astype(np.float32)
    b = (rng.standard_normal(OUT) * 0.01).astype(np.float32)
    got = kernel(x=x, W=W, b=b)
    xm = np.abs(x).mean(dtype=np.float64)
    want = (np.sign(x) @ np.sign(W).T + b) * np.float32(xm)
    err = np.abs(got - want) / (np.abs(want).max())
    print("max rel err:", err.max())
